# revision 1
# baseline (speedup 1.0000x reference)
"""Trainium2 Bass kernel: 3x3 single-channel conv (stride 1, pad 1) on a
4096x4096 fp32 image, sharded over 8 NeuronCores by rows of H.

Numerics: x and w are cast to fp16 on host (tolerance is 2e-2 rel; fp16
single-product gives ~6e-4). conv(x, w) is computed on TensorE as 3
accumulating matmuls per output chunk:
  sum_dj S(w[:, dj]) @ x[:, c+dj : c+dj+cw]
where S(.) is a banded lhsT [128, 128] encoding the three vertical taps
(S[m+di, m] = w[di, dj]) and the horizontal taps come from dj free-dim
offsets of the rhs access pattern. PSUM accumulates in fp32; VectorE adds
bias and downcasts to fp16 on 1024-wide PSUM->SBUF copies (keeping the
scalar engine free to fire output-DMA doorbells); output rides to HBM as
fp16 and is upcast to fp32 on host.

Per core (512 output rows): 4 full tiles of 126 rows + an 8-row tail
computed with 8 column-groups stacked in the partition dim; the tail is
scheduled between full tiles (tail_pos) so its small DMAs hide behind
big-tile compute instead of sitting in the end-of-body drain. Timed
builds unroll several bodies per For_i iteration (trip = reps // unroll)
so input prefetch crosses body boundaries and the all-engine barrier
amortizes.

Measured/core: input ~4.2MB + output ~4.2MB on an additive ~300GB/s DMA
pipe ~= 28us; PE 3 passes ~= 52k cycles + LDWEIGHTS ~= 26us; body ~35us.
"""
import sys
sys.path.insert(0, '/opt/trn_rl_repo')
import numpy as np

import concourse.bass as bass
import concourse.mybir as mybir
from concourse.tile import TileContext
from concourse import bass_utils

H = W = 4096
N_CORES = 8
ROWS_PER_CORE = H // N_CORES          # 512
TILE_OUT = 126                        # clean output rows per 128-row tile
CHUNK = 512                           # matmul moving free dim (one PSUM bank)
N_CHUNKS = W // CHUNK                 # 8
FULL_TILES = ROWS_PER_CORE // TILE_OUT        # 4
TAIL_ROWS = ROWS_PER_CORE - FULL_TILES * TILE_OUT   # 8
WPAD = W + 2                          # 4098
TAIL_G = 8                            # tail column groups
TAIL_GW = W // TAIL_G                 # 512
TAIL_K = TAIL_ROWS + 2                # 10 rows per group
TAIL_STACK = TAIL_G * TAIL_K          # 80 partitions
TAIL_M = TAIL_G * TAIL_ROWS           # 64 psum rows

_cache = {}


def _split_multi_waits(nc):
    """This container's walrus accepts only one sync-wait per instruction;
    Tile's tail drain can carry several. Split extras onto NOPs."""
    ctr = 0
    for f in nc.m.functions:
        for bb in f.blocks:
            new_insts = []
            for ins in bb.instructions:
                si = ins.sync_info
                if si is not None and si.on_wait and len(si.on_wait) > 1:
                    waits = list(si.on_wait)
                    for wt in waits[:-1]:
                        ctr += 1
                        new_insts.append(mybir.InstNoOp(
                            name=f"waitfix_{ctr}",
                            sync_info=mybir.SyncInfo(on_wait=[wt], on_update=[]),
                            bass_nofuse=True,
                            engine=ins.engine,
                        ))
                    si.on_wait = [waits[-1]]
                new_insts.append(ins)
            bb.instructions[:] = new_insts
    return nc


def _build_nc(reps=1, mode="full", out_ring="scalar", order="group4",
              xbounds=(0, 1026, 2562, WPAD), xbufs=4, osplit=2, hint=True,
              psum_bufs=3, copy_eng="vector", unroll=4, obufs=3,
              alt_rings=False,
              tail_pos=2, dma_prio=None, out_prio=None, last_osplit=4,
              gsz=2, span=1, tail_load_t=-1):
    f32 = mybir.dt.float32
    f16 = mybir.dt.float16
    do_pe = mode in ("full", "pe_only", "no_out")
    do_act = mode in ("full", "no_out")
    do_out = mode in ("full", "dma_only", "out_only")
    do_in = mode in ("full", "pe_only", "no_out", "dma_only", "in_only")
    nc = bass.Bass()
    xx_d = nc.dram_tensor("xx", [ROWS_PER_CORE + 2, WPAD], f16,
                          kind="ExternalInput")
    # 3 dj blocks, each a banded lhsT [128, 128] (2 zero cols of padding)
    sm_d = nc.dram_tensor("smat", [128, 3 * 128], f16, kind="ExternalInput")
    # tail: 3 dj blocks, stacked block-diag lhsT [80, 64]
    st_d = nc.dram_tensor("stail", [TAIL_STACK, 3 * TAIL_M], f16,
                          kind="ExternalInput")
    bias_in = nc.dram_tensor("bias_in", [128, 1], f32, kind="ExternalInput")
    y = nc.dram_tensor("y", [ROWS_PER_CORE, W], f16, kind="ExternalOutput")

    with TileContext(nc) as tc:
        tail_banks = 8 - gsz * psum_bufs
        with tc.tile_pool(name="consts", bufs=1) as cpool, \
             tc.tile_pool(name="xt", bufs=xbufs) as xpool, \
             tc.tile_pool(name="ot", bufs=obufs) as opool, \
             tc.tile_pool(name="psum", bufs=psum_bufs, space="PSUM") as ppool, \
             tc.tile_pool(name="psumt", bufs=max(tail_banks, 1),
                          space="PSUM") as ppool_t:
            if tail_banks < 1:
                ppool_t = ppool
            # const loads ride the SWDGE (gpsimd) ring so they never queue
            # ahead of tile 0's input pieces on the SP HWDGE FIFO
            s_t = cpool.tile([128, 3 * 128], f16)
            nc.gpsimd.dma_start(s_t[:], sm_d[:])
            st_t = cpool.tile([TAIL_STACK, 3 * TAIL_M], f16)
            nc.gpsimd.dma_start(st_t[:], st_d[:])
            b_t = cpool.tile([128, 1], f32)
            nc.gpsimd.dma_start(b_t[:], bias_in[:])
            zt = None
            if mode in ("dma_only", "out_only"):
                zt = cpool.tile([128, W], f16)
                nc.gpsimd.memset(zt[:], 0.0)

            out_eng = nc.scalar if out_ring == "scalar" else nc.sync

            def mm_passes(ps_list, src, lhs_tile, mwidth, chunk_ids,
                          cw=CHUNK):
                """Issue 3 dj passes over the given chunks; pass-outer so
                consecutive matmuls reuse one stationary matrix.
                ps_list[i] is the [*, cw] PSUM destination of chunk i."""
                if order == "group4":
                    for dj in range(3):
                        for ci, c0 in enumerate(chunk_ids):
                            nc.tensor.matmul(
                                ps_list[ci],
                                lhs_tile[:, dj * mwidth:(dj + 1) * mwidth],
                                src[:, c0 + dj:c0 + dj + cw],
                                start=(dj == 0), stop=(dj == 2),
                            )
                else:
                    for ci, c0 in enumerate(chunk_ids):
                        for dj in range(3):
                            nc.tensor.matmul(
                                ps_list[ci],
                                lhs_tile[:, dj * mwidth:(dj + 1) * mwidth],
                                src[:, c0 + dj:c0 + dj + cw],
                                start=(dj == 0), stop=(dj == 2),
                            )

            def copy_psum(eng, dst_ap, src_ap, nrows):
                if eng == "act":
                    nc.scalar.activation(
                        dst_ap, src_ap,
                        mybir.ActivationFunctionType.Identity,
                        bias=b_t[:nrows, :], scale=1.0,
                    )
                else:
                    nc.vector.tensor_scalar_add(dst_ap, src_ap,
                                                b_t[:nrows, :])

            def pick_eng(idx):
                if copy_eng == "both":
                    return ("act", "vec")[idx % 2]
                return "act" if copy_eng == "act" else "vec"

            def full_tile(t):
                k = 128
                r0 = t * TILE_OUT
                xx = xpool.tile([128, WPAD], f16, tag="xx")
                in_eng = (nc.sync, nc.scalar)[t % 2] if alt_rings else nc.sync
                if do_in:
                    for i in range(len(xbounds) - 1):
                        lo, hi = xbounds[i], xbounds[i + 1]
                        dd = in_eng.dma_start(xx[:k, lo:hi],
                                              xx_d[r0:r0 + k, lo:hi])
                        if dma_prio is not None:
                            dd.ins.bass_priority = dma_prio
                ot = opool.tile([128, W], f16, tag="ot")
                gw = gsz * CHUNK
                ngroups = N_CHUNKS // gsz
                oeng = (nc.scalar, nc.sync)[t % 2] if alt_rings else out_eng
                osp = last_osplit if t == FULL_TILES - 1 else osplit
                for g0 in range(0, ngroups, span):
                    sgroups = list(range(g0, min(g0 + span, ngroups)))
                    ps_map = {}
                    if do_pe:
                        for g in sgroups:
                            ps_map[g] = ppool.tile([128, gw], f32, tag="ps",
                                                   name=f"ps_t{t}_g{g}")
                        if order == "group4":
                            for dj in range(3):
                                for g in sgroups:
                                    for ci in range(gsz):
                                        c0 = g * gw + ci * CHUNK
                                        nc.tensor.matmul(
                                            ps_map[g][:, ci * CHUNK:
                                                      (ci + 1) * CHUNK],
                                            s_t[:, dj * 128:(dj + 1) * 128],
                                            xx[:, c0 + dj:c0 + dj + CHUNK],
                                            start=(dj == 0), stop=(dj == 2),
                                        )
                        else:
                            for g in sgroups:
                                for ci in range(gsz):
                                    c0 = g * gw + ci * CHUNK
                                    for dj in range(3):
                                        nc.tensor.matmul(
                                            ps_map[g][:, ci * CHUNK:
                                                      (ci + 1) * CHUNK],
                                            s_t[:, dj * 128:(dj + 1) * 128],
                                            xx[:, c0 + dj:c0 + dj + CHUNK],
                                            start=(dj == 0), stop=(dj == 2),
                                        )
                    for g in sgroups:
                        gc0 = g * gw
                        if do_act:
                            copy_psum(pick_eng(t * ngroups + g),
                                      ot[:TILE_OUT, gc0:gc0 + gw],
                                      ps_map[g][:TILE_OUT, :], TILE_OUT)
                        if do_out and osp == 0:
                            src_t = ot if do_act else zt
                            od = oeng.dma_start(
                                y[r0:r0 + TILE_OUT, gc0:gc0 + gw],
                                src_t[:TILE_OUT, gc0:gc0 + gw])
                            if out_prio is not None:
                                od.ins.bass_priority = out_prio
                if do_out and osp > 0:
                    src_t = ot if do_act else zt
                    ow = W // osp
                    for i in range(osp):
                        od = oeng.dma_start(
                            y[r0:r0 + TILE_OUT, i * ow:(i + 1) * ow],
                            src_t[:TILE_OUT, i * ow:(i + 1) * ow])
                        if out_prio is not None:
                            od.ins.bass_priority = out_prio
                if do_out and osp < 0:
                    # row-wise split: each piece is a contiguous DRAM block
                    src_t = ot if do_act else zt
                    nsp = -osp
                    bnds = [TILE_OUT * i // nsp for i in range(nsp + 1)]
                    for i in range(nsp):
                        od = oeng.dma_start(
                            y[r0 + bnds[i]:r0 + bnds[i + 1], :],
                            src_t[bnds[i]:bnds[i + 1], :])
                        if out_prio is not None:
                            od.ins.bass_priority = out_prio

            def tail_load(eng=None):
                r0 = FULL_TILES * TILE_OUT   # shard row 504
                eng = eng if eng is not None else nc.scalar
                xxs = xpool.tile([TAIL_STACK, TAIL_GW + 2], f16, tag="txx")
                if do_in:
                    for g in range(TAIL_G):
                        gc = g * TAIL_GW
                        eng.dma_start(
                            xxs[g * TAIL_K:(g + 1) * TAIL_K, :],
                            xx_d[r0:r0 + TAIL_K, gc:gc + TAIL_GW + 2])
                return xxs[:, :]

            def tail_tile(txx):
                r0 = FULL_TILES * TILE_OUT   # shard row 504
                ot = opool.tile([TAIL_M, TAIL_GW], f16, tag="tot")
                chunk_ids = [c * CHUNK for c in range(TAIL_GW // CHUNK)]
                if do_pe:
                    ps_list = []
                    for ci in range(len(chunk_ids)):
                        if ppool_t is ppool:
                            # tail shares the main pool: reuse the main
                            # tag/shape so no extra slot space is reserved
                            ps_f = ppool.tile([128, gsz * CHUNK], f32,
                                              tag="ps", name=f"ps_tail{ci}")
                            ps_list.append(ps_f[:TAIL_M, :CHUNK])
                        else:
                            ps_i = ppool_t.tile([TAIL_M, CHUNK], f32,
                                                tag="tps")
                            ps_list.append(ps_i[:, :])
                    mm_passes(ps_list, txx, st_t, TAIL_M, chunk_ids)
                if do_act:
                    for ci, c0 in enumerate(chunk_ids):
                        copy_psum(pick_eng(ci), ot[:, c0:c0 + CHUNK],
                                  ps_list[ci], TAIL_M)
                if do_out:
                    src_t = ot if do_act else zt
                    for g in range(TAIL_G):
                        out_eng.dma_start(
                            y[r0:r0 + TAIL_ROWS,
                              g * TAIL_GW:(g + 1) * TAIL_GW],
                            src_t[g * TAIL_ROWS:(g + 1) * TAIL_ROWS,
                                  :TAIL_GW])

            def body():
                # tail_load_t: -1 = body start (scalar ring); t>=0 = issue
                # after full_tile(t) on the sync ring, so tiles 0..t get
                # the pipe first and the tail loads don't sit behind
                # dependency-stalled out-DMAs on the scalar ring
                txx = None
                if tail_load_t < 0:
                    txx = tail_load(nc.scalar)
                if tail_pos == 0:
                    tail_tile(txx)
                for t in range(FULL_TILES):
                    full_tile(t)
                    if t == tail_load_t:
                        txx = tail_load(nc.sync)
                    if t + 1 == tail_pos:
                        tail_tile(txx)
                if tail_pos > FULL_TILES:
                    tail_tile(txx)

            if reps == 1:
                body()
            else:
                # trip count = reps // unroll so `reps` counts BODY
                # executions regardless of unroll (slope stays per-body)
                while reps % unroll:
                    unroll -= 1
                hints = (mybir.EngineType.PE,) if hint else ()
                with tc.For_i(0, reps // unroll, 1, hint_engines=hints):
                    for _ in range(unroll):
                        body()

    _split_multi_waits(nc)
    return nc


def _make_smat(w3):
    """[128, 3*128] fp16: dj-major blocks, each a banded lhsT [128, 128]
    with band weights w[di, dj]; cols 126, 127 are zero."""
    out = np.zeros((128, 3 * 128), dtype=np.float16)
    idx = np.arange(TILE_OUT)
    for dj in range(3):
        blk = out[:, dj * 128:dj * 128 + 128]
        for di in range(3):
            blk[idx + di, idx] = w3[di, dj]
    return out


def _make_stail(w3):
    """[80, 3*64] fp16: block-diagonal stacked tail lhsT per dj."""
    out = np.zeros((TAIL_STACK, 3 * TAIL_M), dtype=np.float16)
    idx = np.arange(TAIL_ROWS)
    for dj in range(3):
        blk = out[:, dj * TAIL_M:(dj + 1) * TAIL_M]
        for g in range(TAIL_G):
            sub = blk[g * TAIL_K:(g + 1) * TAIL_K,
                      g * TAIL_ROWS:(g + 1) * TAIL_ROWS]
            for di in range(3):
                sub[idx + di, idx] = w3[di, dj]
    return out


def kernel(x, weight, bias):
    x = np.asarray(x, dtype=np.float32)
    weight = np.asarray(weight, dtype=np.float32)
    bias = np.asarray(bias, dtype=np.float32)
    w3 = weight.reshape(3, 3).astype(np.float16)

    if "nc" not in _cache:
        _cache["nc"] = _build_nc()
    nc = _cache["nc"]

    xxp = np.zeros((H + 2, WPAD), dtype=np.float16)
    xxp[1:H + 1, 1:W + 1] = x.astype(np.float16)

    smat = _make_smat(w3)
    stail = _make_stail(w3)
    bias_bc = np.full((128, 1), bias[0], dtype=np.float32)

    in_maps = []
    for c in range(N_CORES):
        r0 = c * ROWS_PER_CORE
        in_maps.append({
            "xx": np.ascontiguousarray(xxp[r0:r0 + ROWS_PER_CORE + 2, :]),
            "smat": smat,
            "stail": stail,
            "bias_in": bias_bc,
        })

    _cache["in_maps"] = in_maps
    res = None
    for attempt in range(3):
        try:
            res = bass_utils.run_bass_kernel_spmd(
                nc, in_maps, core_ids=list(range(N_CORES)))
            break
        except Exception:
            if attempt == 2:
                raise
    out = np.empty((H, W), dtype=np.float32)
    for c in range(N_CORES):
        out[c * ROWS_PER_CORE:(c + 1) * ROWS_PER_CORE, :] = \
            res.results[c]["y"].astype(np.float32)
    return out

